# revision 29
# baseline (speedup 1.0000x reference)
# Distributed GQA attention prefill kernel for one TRN2 chip (8 NeuronCores).
#
# Problem: B=2, S=1024, D=2048, H=32 q-heads, KV=4 kv-heads, HD=64, causal,
# RoPE, f32 I/O. Sharding: core d = (batch g=d//4, kv-head kv=d%4). Each core
# computes q-proj for its 8 q heads, k/v-proj for its kv head, attention for
# 8 heads, and the full-channel o_proj for its 512 output columns over 256-row
# blocks, fed by one bf16 AllGather of oT channels per q-block pair (j, j+4)
# across its group of 4 cores.
#
# Key structure (from perfetto-driven tuning):
# - All inputs pre-cast to bf16 on the host; plain HWDGE DMAs, xt first.
# - Attention is software-pipelined per head-quad step: score matmuls + one
#   fused [128,1024] exp (2 PSUM banks) run one step ahead of attn@v, so ACT
#   (the pacing engine at ~1.15us/exp) never waits and PE stays dense.
# - vaug carries 64 ones-rows, so attn@v emits softmax denominators
#   pre-broadcast 64-ways: normalization is one [64,512] copy + one
#   reciprocal_approx_fast + 4 muls per block, no [1,N] single-lane ops.
# - Pairs run light-first (0,1,2,3); AllGathers serialize on the gpsimd
#   collective chain, so o_proj(pair k-2) is emitted after AG(k) triggers
#   (its gather is then provably complete) and the last AG overlaps the two
#   remaining o_projs.
import sys

import numpy as np

try:
    import concourse.bass as bass  # noqa: F401
except ImportError:
    for p in ("/opt/trn_rl_repo", "/root/.axon_site/_ro/trn_rl_repo"):
        if p not in sys.path:
            sys.path.append(p)
    import concourse.bass as bass  # noqa: F401

import concourse.bacc as bacc
import concourse.mybir as mybir
import concourse.tile as tile
from concourse import masks
from concourse.bass_utils import run_bass_kernel_spmd

S = 1024
D = 2048
H = 32
KV = 4
HD = 64
NH = 8  # q heads per core
P = 128
SC = S // P  # 8 seq chunks
DC = D // P  # 16 D chunks
N_CORES = 8
GROUPS = [[0, 1, 2, 3], [4, 5, 6, 7]]

F32 = mybir.dt.float32
BF16 = mybir.dt.bfloat16

_NC_CACHE = {}


def _build_graph():
    nc = bacc.Bacc("TRN2", target_bir_lowering=False, debug=False, num_devices=N_CORES)

    xt_p = nc.dram_tensor("xt", [SC, P, DC * P], BF16, kind="ExternalInput")
    wq_p = nc.dram_tensor("wq", [P, DC * 512], BF16, kind="ExternalInput")
    wkv_p = nc.dram_tensor("wkv", [P, DC * 2 * HD], BF16, kind="ExternalInput")
    wo_p = nc.dram_tensor("wo", [P, DC * 512], BF16, kind="ExternalInput")
    cs_p = nc.dram_tensor("cs8", [P, SC * 256], BF16, kind="ExternalInput")
    sn_p = nc.dram_tensor("sn8", [P, SC * 256], BF16, kind="ExternalInput")
    mk_p = nc.dram_tensor("mkb", [P, SC * 512], BF16, kind="ExternalInput")
    out_p = nc.dram_tensor("out", [S, 512], F32, kind="ExternalOutput")

    with tile.TileContext(nc) as tc:
        with (
            tc.tile_pool(name="const", bufs=1) as constp,
            tc.tile_pool(name="big", bufs=1) as bigp,
            tc.tile_pool(name="work", bufs=1) as workp,
            tc.tile_pool(name="rt", bufs=4) as rtp,
            tc.tile_pool(name="attn", bufs=3) as attnp,
            tc.tile_pool(name="opart", bufs=2) as opartp,
            tc.tile_pool(name="tiny", bufs=1) as tinyp,
            tc.tile_pool(name="psum", bufs=1, space="PSUM") as psump,
            tc.tile_pool(name="dram", bufs=1, space="DRAM") as dramp,
        ):
            # ---- constants (cheap engine work first; DMAs ordered by need) ----
            ident = constp.tile([P, P], BF16, tag="ident")
            masks.make_identity(nc, ident[:])

            # Bulk loads: inputs are pre-cast to bf16 on the host, so these
            # are plain direct DMAs on the two HWDGE rings — no staging.
            xT_all = bigp.tile([P, SC * DC * P], BF16, tag="xT_all")
            wkv_all = bigp.tile([P, DC * 2 * HD], BF16, tag="wkv_all")
            wq_all = bigp.tile([P, DC * 512], BF16, tag="wq_all")
            wo_all = bigp.tile([P, DC * 512], BF16, tag="wo_all")

            def load_xt(s, eng):
                h = D // 2
                eng.dma_start(
                    out=xT_all[:, s * D : s * D + h], in_=xt_p[s, :, 0:h]
                )
                eng.dma_start(
                    out=xT_all[:, s * D + h : (s + 1) * D], in_=xt_p[s, :, h:D]
                )

            def load_wq(g, eng):
                eng.dma_start(
                    out=wq_all[:, g * 2048 : (g + 1) * 2048],
                    in_=wq_p[:, g * 2048 : (g + 1) * 2048],
                )

            def load_wo(g, eng):
                eng.dma_start(
                    out=wo_all[:, g * 2048 : (g + 1) * 2048],
                    in_=wo_p[:, g * 2048 : (g + 1) * 2048],
                )

            cst = constp.tile([P, SC * 256], BF16, tag="cst")  # cos, tiled x8 heads
            snt = constp.tile([P, SC * 256], BF16, tag="snt")
            mkt = constp.tile([P, SC * 512], BF16, tag="mkt")  # binary diag masks^T
            nc.scalar.dma_start(out=wkv_all[:], in_=wkv_p[:, :])
            for s in range(SC):
                load_xt(s, nc.sync if s % 2 == 0 else nc.scalar)
            nc.sync.dma_start(out=cst[:], in_=cs_p[:, :])
            nc.scalar.dma_start(out=snt[:], in_=sn_p[:, :])
            for g in range(4):
                load_wq(g, nc.sync if g % 2 == 0 else nc.scalar)
            nc.sync.dma_start(out=mkt[:], in_=mk_p[:, :])
            for g in range(4):
                load_wo(g, nc.scalar if g % 2 == 0 else nc.sync)

            # ---- projections + RoPE (natural [s, ch] layout) ----
            qrot = [workp.tile([P, NH * HD], BF16, tag=f"qr{s}", name=f"qr{s}") for s in range(SC)]
            krot = [workp.tile([P, HD], BF16, tag=f"kr{s}", name=f"kr{s}") for s in range(SC)]
            vaug = [workp.tile([P, 2 * HD], BF16, tag=f"va{s}", name=f"va{s}") for s in range(SC)]

            def rope(ps_ap, dst, s, nh):
                # ps_ap: PSUM AP [128, nh*64] f32; dst: SBUF bf16 same shape
                pv = ps_ap.rearrange("p (h t c) -> p h t c", h=nh, t=2)
                dv = dst[:].rearrange("p (h t c) -> p h t c", h=nh, t=2)
                cs = cst[:, s * 256 : s * 256 + nh * 32].rearrange(
                    "p (h c) -> p h c", h=nh
                )
                sn = snt[:, s * 256 : s * 256 + nh * 32].rearrange(
                    "p (h c) -> p h c", h=nh
                )
                lo, hi = pv[:, :, 0, :], pv[:, :, 1, :]
                t1 = rtp.tile([P, NH * 32], F32, tag="rt1")
                t2 = rtp.tile([P, NH * 32], F32, tag="rt2")
                t1v = t1[:, : nh * 32].rearrange("p (h c) -> p h c", h=nh)
                t2v = t2[:, : nh * 32].rearrange("p (h c) -> p h c", h=nh)
                nc.any.tensor_mul(t1v, lo, cs)
                nc.any.tensor_mul(t2v, hi, sn)
                nc.any.tensor_sub(dv[:, :, 0, :], t1v, t2v)
                nc.any.tensor_mul(t1v, hi, cs)
                nc.any.tensor_mul(t2v, lo, sn)
                nc.any.tensor_add(dv[:, :, 1, :], t1v, t2v)

            qTall = workp.tile([64, NH * S], BF16, tag="qTall")
            kT = workp.tile([64, S], BF16, tag="kT")

            def emit_kv(s):
                pkv_t = psump.tile([P, 1024], F32, tag="big", bufs=2, name=f"pkv{s}")
                pkv = pkv_t[:, 0 : 2 * HD]
                for d in range(DC):
                    nc.tensor.matmul(
                        pkv,
                        xT_all[:, s * D + d * P : s * D + (d + 1) * P],
                        wkv_all[:, d * 2 * HD : (d + 1) * 2 * HD],
                        start=(d == 0),
                        stop=(d == DC - 1),
                    )
                rope(pkv[:, 0:HD], krot[s], s, 1)
                nc.vector.tensor_copy(vaug[s][:, 0:HD], pkv[:, HD : 2 * HD])
                nc.any.memset(vaug[s][:, HD : 2 * HD], 1.0)
                tpk = psump.tile([64, P], BF16, tag="oa", bufs=2, name=f"tpk{s}")
                nc.tensor.transpose(tpk[:], krot[s][:], ident[:])
                nc.vector.tensor_copy(kT[:, s * P : (s + 1) * P], tpk[:])

            def emit_q(s):
                pq_t = psump.tile([P, 1024], F32, tag="big", bufs=2, name=f"pq{s}")
                pq = pq_t[:, 0 : NH * HD]
                for d in range(DC):
                    nc.tensor.matmul(
                        pq,
                        xT_all[:, s * D + d * P : s * D + (d + 1) * P],
                        wq_all[:, d * 512 : (d + 1) * 512],
                        start=(d == 0),
                        stop=(d == DC - 1),
                    )
                rope(pq[:, :], qrot[s], s, NH)
                for h in range(NH):
                    tpq = psump.tile([64, P], BF16, tag="oa", bufs=2, name=f"tpq{s}_{h}")
                    nc.tensor.transpose(
                        tpq[:], qrot[s][:, h * HD : (h + 1) * HD], ident[:]
                    )
                    nc.vector.tensor_copy(
                        qTall[:, h * S + s * P : h * S + (s + 1) * P], tpq[:]
                    )

            for s in range(SC):
                emit_kv(s)
            for s in range(SC):
                emit_q(s)

            # view: [64, a(4), c(2), h(8), i(128)]; a scores matmul's rhs takes
            # (c, h, i)-ordered columns: [j:h | j:h' | j+4:h | j+4:h']
            qview = qTall[:].rearrange("p (h c a b) -> p a c h b", h=NH, c=2, a=4, b=P)

            # oTp[j][c]: pair-j output tile for channel group c (2 heads),
            # cols = [block j (128) | block j+4 (128)] -> contiguous AG ship.
            oTp = [
                [
                    workp.tile([P, 256], BF16, tag=f"oTp{j}c{c}", name=f"oTp{j}c{c}")
                    for c in range(4)
                ]
                for j in range(4)
            ]
            agin = [dramp.tile([512, 256], BF16, name=f"agin{j}") for j in range(4)]
            agout = [dramp.tile([D, 256], BF16, name=f"agout{j}") for j in range(4)]

            def emit_av_norm(j, qd, ats):
                """attn@v for quad (j, qd) into one fused [128, 1024] PSUM
                (block j in cols 0:512, block j+4 in 512:1024). Rows 0:64 are
                o, rows 64:128 are the softmax denominators pre-broadcast 64
                ways (vaug cols 64:128 are ones), so normalization needs no
                [1,N] single-lane ops and no broadcast matmul."""
                oa = psump.tile([P, 1024], F32, tag="oa", bufs=2)
                for skc in range(j + 1):  # block j accumulation
                    nc.tensor.matmul(
                        oa[:, 0:512],
                        vaug[skc][:],
                        ats[skc][:, 0:512],
                        start=(skc == 0),
                        stop=(skc == j),
                        skip_group_check=True,
                    )
                for skc in range(j + 5):  # block j+4 accumulation
                    if skc <= j:
                        rhs = ats[skc][:, 512:1024]
                    else:
                        si, half = divmod(skc - j - 1, 2)
                        rhs = ats[j + 1 + si][:, half * 512 : half * 512 + 512]
                    nc.tensor.matmul(
                        oa[:, 512:1024],
                        vaug[skc][:],
                        rhs,
                        start=(skc == 0),
                        stop=(skc == j + 4),
                        skip_group_check=True,
                    )
                # per-half normalization chains (block a overlaps block b's AV)
                for bi, qb in enumerate((j, j + 4)):
                    cols = slice(bi * 512, bi * 512 + 512)
                    rec = tinyp.tile([64, 512], F32, tag=f"rec{bi}")
                    nc.vector.tensor_copy(rec[:], oa[HD : 2 * HD, cols])
                    rc2 = tinyp.tile([64, 512], F32, tag=f"rc2{bi}")
                    nc.vector.reciprocal_approx_fast(rc2[:], rec[:])
                    for h4 in range(4):
                        h = 4 * qd + h4
                        nc.vector.tensor_mul(
                            oTp[j][h // 2][
                                64 * (h % 2) : 64 * (h % 2) + 64,
                                bi * P : (bi + 1) * P,
                            ],
                            oa[0:HD, bi * 512 + h4 * P : bi * 512 + (h4 + 1) * P],
                            rc2[:, h4 * P : (h4 + 1) * P],
                        )
                # ship the two channel groups this quad completed
                for c in (2 * qd, 2 * qd + 1):
                    nc.scalar.dma_start(
                        out=agin[j][c * P : (c + 1) * P, :], in_=oTp[j][c][:]
                    )
                if qd == 1:
                    nc.gpsimd.collective_compute(
                        "AllGather",
                        mybir.AluOpType.bypass,
                        replica_groups=GROUPS,
                        ins=[agin[j].opt()],
                        outs=[agout[j].opt()],
                    )

            def emit_scores(j, qd):
                """score matmuls + fused exps for quad (j, qd); returns the
                list of at tiles: ats[0..j] are both-tiles, ats[j+1], ats[j+2]
                are the fused single pairs (skc j+1/j+2 and j+3/j+4)."""
                quad = qview[:, j, 0, 4 * qd : 4 * qd + 4, :]
                quad_b = qview[:, j, 1, 4 * qd : 4 * qd + 4, :]
                ats = []
                for skc in range(j + 1):
                    sc2 = psump.tile([P, 1024], F32, tag="big", bufs=2)
                    nc.tensor.matmul(
                        sc2[:, 0:512],
                        kT[:, skc * P : (skc + 1) * P],
                        quad,
                        start=True,
                        stop=True,
                    )
                    nc.tensor.matmul(
                        sc2[:, 512:1024],
                        kT[:, skc * P : (skc + 1) * P],
                        quad_b,
                        start=True,
                        stop=True,
                    )
                    at2 = attnp.tile([P, 1024], BF16, tag="at", bufs=12)
                    nc.scalar.activation(
                        at2[:], sc2[:], mybir.ActivationFunctionType.Exp, scale=0.125
                    )
                    if skc == j:
                        nc.vector.tensor_mul(
                            at2[:, 0:512],
                            at2[:, 0:512],
                            mkt[:, j * 512 : (j + 1) * 512],
                        )
                    ats.append(at2)
                for si in range(2):
                    skc0, skc1 = j + 1 + 2 * si, j + 2 + 2 * si
                    sc2 = psump.tile([P, 1024], F32, tag="big", bufs=2)
                    nc.tensor.matmul(
                        sc2[:, 0:512],
                        kT[:, skc0 * P : (skc0 + 1) * P],
                        quad_b,
                        start=True,
                        stop=True,
                    )
                    nc.tensor.matmul(
                        sc2[:, 512:1024],
                        kT[:, skc1 * P : (skc1 + 1) * P],
                        quad_b,
                        start=True,
                        stop=True,
                    )
                    at2 = attnp.tile([P, 1024], BF16, tag="at", bufs=12)
                    nc.scalar.activation(
                        at2[:], sc2[:], mybir.ActivationFunctionType.Exp, scale=0.125
                    )
                    if si == 1:
                        nc.vector.tensor_mul(
                            at2[:, 512:1024],
                            at2[:, 512:1024],
                            mkt[:, (j + 4) * 512 : (j + 5) * 512],
                        )
                    ats.append(at2)
                return ats

            def emit_oproj(j):
                # o_proj for pair j from the AllGathered full-channel oT.
                # Readback split in 4 chunk-DMAs on both rings; the two srow
                # PSUM accumulators advance chunk-by-chunk as data lands.
                ag_sb = opartp.tile([P, DC * 256], BF16, tag="agsb", bufs=2)
                for g in range(4):
                    eng = nc.sync if g % 2 == 0 else nc.scalar
                    eng.dma_start(
                        out=ag_sb[:, g * 1024 : (g + 1) * 1024].rearrange(
                            "p (c n) -> p c n", c=4
                        ),
                        in_=agout[j][g * 512 : (g + 1) * 512, :].rearrange(
                            "(c p) n -> p c n", p=P
                        ),
                    )
                po = psump.tile([P, 1024], F32, tag="big", bufs=2, name=f"po{j}")
                for c16 in range(DC):
                    for srow in range(2):
                        nc.tensor.matmul(
                            po[:, srow * 512 : (srow + 1) * 512],
                            ag_sb[:, c16 * 256 + srow * P : c16 * 256 + (srow + 1) * P],
                            wo_all[:, c16 * 512 : (c16 + 1) * 512],
                            start=(c16 == 0),
                            stop=(c16 == DC - 1),
                        )
                for srow in range(2):
                    osb = opartp.tile([P, 512], F32, tag="osb", bufs=3)
                    nc.vector.tensor_copy(osb[:], po[:, srow * 512 : (srow + 1) * 512])
                    nc.sync.dma_start(
                        out=out_p[256 * j + srow * P : 256 * j + (srow + 1) * P, :],
                        in_=osb[:],
                    )

            # ---- software-pipelined attention: scores/exp run one quad
            # ahead of attn@v+normalization so ACT (the pacing engine) never
            # waits on PE and PE stays dense. Pairs run light-first so the
            # serialized AllGather chain starts as early as possible; after
            # each AG trigger the PREVIOUS pair's o_proj is emitted (the
            # gpsimd collective chain guarantees its gather has completed).
            pair_order = (0, 1, 2, 3)
            steps = [(j, qd) for j in pair_order for qd in range(2)]
            prev = None
            for idx, (j, qd) in enumerate(steps):
                if prev is not None:
                    emit_av_norm(*prev)
                    pj, pqd = prev[0], prev[1]
                    k = pair_order.index(pj)
                    if pqd == 1 and k >= 2:
                        emit_oproj(pair_order[k - 2])
                ats = emit_scores(j, qd)
                prev = (j, qd, ats)
            emit_av_norm(*prev)
            for k in (-3, -2, -1):
                emit_oproj(pair_order[k])

    nc.compile()
    return nc


def _get_nc():
    if "nc" not in _NC_CACHE:
        _NC_CACHE["nc"] = _build_graph()
    return _NC_CACHE["nc"]


def _shard_inputs(x, wq, wk, wv, wo, cos, sin, mask, pos):
    import ml_dtypes

    bf16 = ml_dtypes.bfloat16
    x = np.asarray(x, dtype=np.float32).astype(bf16)
    wq = np.asarray(wq, dtype=np.float32).astype(bf16)
    wk = np.asarray(wk, dtype=np.float32).astype(bf16)
    wv = np.asarray(wv, dtype=np.float32).astype(bf16)
    wo = np.asarray(wo, dtype=np.float32).astype(bf16)
    cos = np.asarray(cos, dtype=np.float32)
    sin = np.asarray(sin, dtype=np.float32)
    mask = np.asarray(mask, dtype=np.float32)
    p = int(pos)

    def pblock(a, nchunks):
        # [(chunks*128), n] -> [128, chunks, n] -> [128, chunks*n]
        n = a.shape[1]
        return np.ascontiguousarray(
            a.reshape(nchunks, P, n).transpose(1, 0, 2).reshape(P, nchunks * n)
        )

    cs = cos[p : p + S]  # [S, 32]
    sn = sin[p : p + S]
    cs8 = pblock(np.tile(cs, (1, NH)), SC).astype(bf16)  # [128, 8*256]
    sn8 = pblock(np.tile(sn, (1, NH)), SC).astype(bf16)
    # transposed diagonal 128x128 blocks of the additive mask, pre-scaled by
    # sqrt(HD) so exp(scale*(scores + 8*mask)) == exp(scores/8 + mask)
    mkb = np.concatenate(
        [
            np.tile(
                (mask[j * P : (j + 1) * P, j * P : (j + 1) * P].T >= -0.5).astype(
                    bf16
                ),
                (1, 4),
            )
            for j in range(SC)
        ],
        axis=1,
    )
    mkb = np.ascontiguousarray(mkb)  # [128, 8*512], diag blocks tiled x4 heads

    in_maps = []
    for d in range(N_CORES):
        g, kv = d // 4, d % 4
        in_maps.append(
            {
                "xt": np.ascontiguousarray(
                    x[g].T.reshape(DC, P, SC, P).transpose(2, 1, 0, 3).reshape(SC, P, D)
                ),
                "wq": pblock(wq[:, kv * 512 : (kv + 1) * 512], DC),
                "wkv": pblock(
                    np.concatenate(
                        [
                            wk[:, kv * HD : (kv + 1) * HD],
                            wv[:, kv * HD : (kv + 1) * HD],
                        ],
                        axis=1,
                    ),
                    DC,
                ),
                "wo": pblock(wo[:, kv * 512 : (kv + 1) * 512], DC),
                "cs8": cs8,
                "sn8": sn8,
                "mkb": mkb,
            }
        )
    return in_maps


def _run(inputs, trace=False, trace_kwargs=None):
    nc = _get_nc()
    in_maps = _shard_inputs(**inputs)
    res = run_bass_kernel_spmd(
        nc,
        in_maps,
        core_ids=list(range(N_CORES)),
        trace=trace,
        **(trace_kwargs or {}),
    )
    B = 2
    out = np.empty((B, S, D), dtype=np.float32)
    for d in range(N_CORES):
        g, kv = d // 4, d % 4
        core_out = res.results[d]["out"]  # [1024, 512]; rows 256j.. = pair j
        cols = slice(kv * 512, (kv + 1) * 512)
        for j in range(4):
            out[g, j * P : (j + 1) * P, cols] = core_out[256 * j : 256 * j + P]
            out[g, (j + 4) * P : (j + 5) * P, cols] = core_out[
                256 * j + P : 256 * j + 256
            ]
    return out, res


def kernel(**inputs) -> np.ndarray:
    out, _ = _run(inputs, trace=False)
    return out



# revision 30
# speedup vs baseline: 1.0340x; 1.0340x over previous
# Distributed GQA attention prefill kernel for one TRN2 chip (8 NeuronCores).
#
# Problem: B=2, S=1024, D=2048, H=32 q-heads, KV=4 kv-heads, HD=64, causal,
# RoPE, f32 I/O. Sharding: core d = (batch g=d//4, kv-head kv=d%4). Each core
# computes q-proj for its 8 q heads, k/v-proj for its kv head, attention for
# 8 heads, and the full-channel o_proj for its 512 output columns over 256-row
# blocks, fed by one bf16 AllGather of oT channels per q-block pair (j, j+4)
# across its group of 4 cores.
#
# Key structure (from perfetto-driven tuning):
# - All inputs pre-cast to bf16 on the host; plain HWDGE DMAs, xt first.
# - Attention is software-pipelined per head-quad step: score matmuls + one
#   fused [128,1024] exp (2 PSUM banks) run one step ahead of attn@v, so ACT
#   (the pacing engine at ~1.15us/exp) never waits and PE stays dense.
# - vaug carries 64 ones-rows, so attn@v emits softmax denominators
#   pre-broadcast 64-ways: normalization is one [64,512] copy + one
#   reciprocal_approx_fast + 4 muls per block, no [1,N] single-lane ops.
# - Pairs run light-first (0,1,2,3); AllGathers serialize on the gpsimd
#   collective chain, so o_proj(pair k-2) is emitted after AG(k) triggers
#   (its gather is then provably complete) and the last AG overlaps the two
#   remaining o_projs.
import sys

import numpy as np

try:
    import concourse.bass as bass  # noqa: F401
except ImportError:
    for p in ("/opt/trn_rl_repo", "/root/.axon_site/_ro/trn_rl_repo"):
        if p not in sys.path:
            sys.path.append(p)
    import concourse.bass as bass  # noqa: F401

import concourse.bacc as bacc
import concourse.mybir as mybir
import concourse.tile as tile
from concourse import masks
from concourse.bass_utils import run_bass_kernel_spmd

S = 1024
D = 2048
H = 32
KV = 4
HD = 64
NH = 8  # q heads per core
P = 128
SC = S // P  # 8 seq chunks
DC = D // P  # 16 D chunks
N_CORES = 8
GROUPS = [[0, 1, 2, 3], [4, 5, 6, 7]]

F32 = mybir.dt.float32
BF16 = mybir.dt.bfloat16

_NC_CACHE = {}


def _build_graph():
    nc = bacc.Bacc("TRN2", target_bir_lowering=False, debug=False, num_devices=N_CORES)

    xt_p = nc.dram_tensor("xt", [SC, P, DC * P], BF16, kind="ExternalInput")
    wq_p = nc.dram_tensor("wq", [P, DC * 512], BF16, kind="ExternalInput")
    wkv_p = nc.dram_tensor("wkv", [P, DC * 2 * HD], BF16, kind="ExternalInput")
    wo_p = nc.dram_tensor("wo", [P, DC * 512], BF16, kind="ExternalInput")
    cs_p = nc.dram_tensor("cs8", [P, SC * 256], BF16, kind="ExternalInput")
    sn_p = nc.dram_tensor("sn8", [P, SC * 256], BF16, kind="ExternalInput")
    mk_p = nc.dram_tensor("mkb", [P, SC * 512], BF16, kind="ExternalInput")
    out_p = nc.dram_tensor("out", [S, 512], F32, kind="ExternalOutput")

    with tile.TileContext(nc) as tc:
        with (
            tc.tile_pool(name="const", bufs=1) as constp,
            tc.tile_pool(name="big", bufs=1) as bigp,
            tc.tile_pool(name="work", bufs=1) as workp,
            tc.tile_pool(name="rt", bufs=4) as rtp,
            tc.tile_pool(name="attn", bufs=3) as attnp,
            tc.tile_pool(name="opart", bufs=2) as opartp,
            tc.tile_pool(name="tiny", bufs=1) as tinyp,
            tc.tile_pool(name="psum", bufs=1, space="PSUM") as psump,
            tc.tile_pool(name="dram", bufs=1, space="DRAM") as dramp,
        ):
            # ---- constants (cheap engine work first; DMAs ordered by need) ----
            ident = constp.tile([P, P], BF16, tag="ident")
            masks.make_identity(nc, ident[:])

            # Bulk loads: inputs are pre-cast to bf16 on the host, so these
            # are plain direct DMAs on the two HWDGE rings — no staging.
            xT_all = bigp.tile([P, SC * DC * P], BF16, tag="xT_all")
            wkv_all = bigp.tile([P, DC * 2 * HD], BF16, tag="wkv_all")
            wq_all = bigp.tile([P, DC * 512], BF16, tag="wq_all")
            wo_all = bigp.tile([P, DC * 512], BF16, tag="wo_all")

            def load_xt(s, eng):
                eng.dma_start(
                    out=xT_all[:, s * D : (s + 1) * D], in_=xt_p[s, :, :]
                )

            def load_wq(g, eng):
                eng.dma_start(
                    out=wq_all[:, g * 2048 : (g + 1) * 2048],
                    in_=wq_p[:, g * 2048 : (g + 1) * 2048],
                )

            def load_wo(g, eng):
                eng.dma_start(
                    out=wo_all[:, g * 2048 : (g + 1) * 2048],
                    in_=wo_p[:, g * 2048 : (g + 1) * 2048],
                )

            cst = constp.tile([P, SC * 256], BF16, tag="cst")  # cos, tiled x8 heads
            snt = constp.tile([P, SC * 256], BF16, tag="snt")
            mkt = constp.tile([P, SC * 512], BF16, tag="mkt")  # binary diag masks^T
            nc.scalar.dma_start(out=wkv_all[:], in_=wkv_p[:, :])
            for s in range(SC):
                load_xt(s, nc.sync if s % 2 == 0 else nc.scalar)
            nc.sync.dma_start(out=cst[:], in_=cs_p[:, :])
            nc.scalar.dma_start(out=snt[:], in_=sn_p[:, :])
            for g in range(4):
                load_wq(g, nc.sync if g % 2 == 0 else nc.scalar)
            nc.sync.dma_start(out=mkt[:], in_=mk_p[:, :])
            for g in range(4):
                load_wo(g, nc.scalar if g % 2 == 0 else nc.sync)

            # ---- projections + RoPE (natural [s, ch] layout) ----
            qrot = [workp.tile([P, NH * HD], BF16, tag=f"qr{s}", name=f"qr{s}") for s in range(SC)]
            krot = [workp.tile([P, HD], BF16, tag=f"kr{s}", name=f"kr{s}") for s in range(SC)]
            vaug = [workp.tile([P, 2 * HD], BF16, tag=f"va{s}", name=f"va{s}") for s in range(SC)]

            def rope(ps_ap, dst, s, nh):
                # ps_ap: PSUM AP [128, nh*64] f32; dst: SBUF bf16 same shape
                pv = ps_ap.rearrange("p (h t c) -> p h t c", h=nh, t=2)
                dv = dst[:].rearrange("p (h t c) -> p h t c", h=nh, t=2)
                cs = cst[:, s * 256 : s * 256 + nh * 32].rearrange(
                    "p (h c) -> p h c", h=nh
                )
                sn = snt[:, s * 256 : s * 256 + nh * 32].rearrange(
                    "p (h c) -> p h c", h=nh
                )
                lo, hi = pv[:, :, 0, :], pv[:, :, 1, :]
                t1 = rtp.tile([P, NH * 32], F32, tag="rt1")
                t2 = rtp.tile([P, NH * 32], F32, tag="rt2")
                t1v = t1[:, : nh * 32].rearrange("p (h c) -> p h c", h=nh)
                t2v = t2[:, : nh * 32].rearrange("p (h c) -> p h c", h=nh)
                nc.any.tensor_mul(t1v, lo, cs)
                nc.any.tensor_mul(t2v, hi, sn)
                nc.any.tensor_sub(dv[:, :, 0, :], t1v, t2v)
                nc.any.tensor_mul(t1v, hi, cs)
                nc.any.tensor_mul(t2v, lo, sn)
                nc.any.tensor_add(dv[:, :, 1, :], t1v, t2v)

            qTall = workp.tile([64, NH * S], BF16, tag="qTall")
            kT = workp.tile([64, S], BF16, tag="kT")

            def emit_kv(s):
                pkv_t = psump.tile([P, 1024], F32, tag="big", bufs=2, name=f"pkv{s}")
                pkv = pkv_t[:, 0 : 2 * HD]
                for d in range(DC):
                    nc.tensor.matmul(
                        pkv,
                        xT_all[:, s * D + d * P : s * D + (d + 1) * P],
                        wkv_all[:, d * 2 * HD : (d + 1) * 2 * HD],
                        start=(d == 0),
                        stop=(d == DC - 1),
                    )
                rope(pkv[:, 0:HD], krot[s], s, 1)
                nc.vector.tensor_copy(vaug[s][:, 0:HD], pkv[:, HD : 2 * HD])
                nc.any.memset(vaug[s][:, HD : 2 * HD], 1.0)
                tpk = psump.tile([64, P], BF16, tag="oa", bufs=2, name=f"tpk{s}")
                nc.tensor.transpose(tpk[:], krot[s][:], ident[:])
                nc.vector.tensor_copy(kT[:, s * P : (s + 1) * P], tpk[:])

            def emit_q(s):
                pq_t = psump.tile([P, 1024], F32, tag="big", bufs=2, name=f"pq{s}")
                pq = pq_t[:, 0 : NH * HD]
                for d in range(DC):
                    nc.tensor.matmul(
                        pq,
                        xT_all[:, s * D + d * P : s * D + (d + 1) * P],
                        wq_all[:, d * 512 : (d + 1) * 512],
                        start=(d == 0),
                        stop=(d == DC - 1),
                    )
                rope(pq[:, :], qrot[s], s, NH)
                for h in range(NH):
                    tpq = psump.tile([64, P], BF16, tag="oa", bufs=2, name=f"tpq{s}_{h}")
                    nc.tensor.transpose(
                        tpq[:], qrot[s][:, h * HD : (h + 1) * HD], ident[:]
                    )
                    nc.vector.tensor_copy(
                        qTall[:, h * S + s * P : h * S + (s + 1) * P], tpq[:]
                    )

            for s in range(SC):
                emit_kv(s)
            for s in range(SC):
                emit_q(s)

            # view: [64, a(4), c(2), h(8), i(128)]; a scores matmul's rhs takes
            # (c, h, i)-ordered columns: [j:h | j:h' | j+4:h | j+4:h']
            qview = qTall[:].rearrange("p (h c a b) -> p a c h b", h=NH, c=2, a=4, b=P)

            # oTp[j][c]: pair-j output tile for channel group c (2 heads),
            # cols = [block j (128) | block j+4 (128)] -> contiguous AG ship.
            oTp = [
                [
                    workp.tile([P, 256], BF16, tag=f"oTp{j}c{c}", name=f"oTp{j}c{c}")
                    for c in range(4)
                ]
                for j in range(4)
            ]
            agin = [dramp.tile([512, 256], BF16, name=f"agin{j}") for j in range(4)]
            agout = [dramp.tile([D, 256], BF16, name=f"agout{j}") for j in range(4)]

            def emit_av_norm(j, qd, ats):
                """attn@v for quad (j, qd) into one fused [128, 1024] PSUM
                (block j in cols 0:512, block j+4 in 512:1024). Rows 0:64 are
                o, rows 64:128 are the softmax denominators pre-broadcast 64
                ways (vaug cols 64:128 are ones), so normalization needs no
                [1,N] single-lane ops and no broadcast matmul."""
                oa = psump.tile([P, 1024], F32, tag="oa", bufs=2)
                for skc in range(j + 1):  # block j accumulation
                    nc.tensor.matmul(
                        oa[:, 0:512],
                        vaug[skc][:],
                        ats[skc][:, 0:512],
                        start=(skc == 0),
                        stop=(skc == j),
                        skip_group_check=True,
                    )
                for skc in range(j + 5):  # block j+4 accumulation
                    if skc <= j:
                        rhs = ats[skc][:, 512:1024]
                    else:
                        si, half = divmod(skc - j - 1, 2)
                        rhs = ats[j + 1 + si][:, half * 512 : half * 512 + 512]
                    nc.tensor.matmul(
                        oa[:, 512:1024],
                        vaug[skc][:],
                        rhs,
                        start=(skc == 0),
                        stop=(skc == j + 4),
                        skip_group_check=True,
                    )
                # per-half normalization chains (block a overlaps block b's AV)
                for bi, qb in enumerate((j, j + 4)):
                    cols = slice(bi * 512, bi * 512 + 512)
                    rec = tinyp.tile([64, 512], F32, tag=f"rec{bi}")
                    nc.vector.tensor_copy(rec[:], oa[HD : 2 * HD, cols])
                    rc2 = tinyp.tile([64, 512], F32, tag=f"rc2{bi}")
                    nc.vector.reciprocal_approx_fast(rc2[:], rec[:])
                    for h4 in range(4):
                        h = 4 * qd + h4
                        nc.vector.tensor_mul(
                            oTp[j][h // 2][
                                64 * (h % 2) : 64 * (h % 2) + 64,
                                bi * P : (bi + 1) * P,
                            ],
                            oa[0:HD, bi * 512 + h4 * P : bi * 512 + (h4 + 1) * P],
                            rc2[:, h4 * P : (h4 + 1) * P],
                        )
                # ship the two channel groups this quad completed
                for c in (2 * qd, 2 * qd + 1):
                    nc.scalar.dma_start(
                        out=agin[j][c * P : (c + 1) * P, :], in_=oTp[j][c][:]
                    )
                if qd == 1:
                    nc.gpsimd.collective_compute(
                        "AllGather",
                        mybir.AluOpType.bypass,
                        replica_groups=GROUPS,
                        ins=[agin[j].opt()],
                        outs=[agout[j].opt()],
                    )

            def emit_scores(j, qd):
                """score matmuls + fused exps for quad (j, qd); returns the
                list of at tiles: ats[0..j] are both-tiles, ats[j+1], ats[j+2]
                are the fused single pairs (skc j+1/j+2 and j+3/j+4)."""
                quad = qview[:, j, 0, 4 * qd : 4 * qd + 4, :]
                quad_b = qview[:, j, 1, 4 * qd : 4 * qd + 4, :]
                ats = []
                for skc in range(j + 1):
                    sc2 = psump.tile([P, 1024], F32, tag="big", bufs=2)
                    nc.tensor.matmul(
                        sc2[:, 0:512],
                        kT[:, skc * P : (skc + 1) * P],
                        quad,
                        start=True,
                        stop=True,
                    )
                    nc.tensor.matmul(
                        sc2[:, 512:1024],
                        kT[:, skc * P : (skc + 1) * P],
                        quad_b,
                        start=True,
                        stop=True,
                    )
                    at2 = attnp.tile([P, 1024], BF16, tag="at", bufs=12)
                    nc.scalar.activation(
                        at2[:], sc2[:], mybir.ActivationFunctionType.Exp, scale=0.125
                    )
                    if skc == j:
                        nc.vector.tensor_mul(
                            at2[:, 0:512],
                            at2[:, 0:512],
                            mkt[:, j * 512 : (j + 1) * 512],
                        )
                    ats.append(at2)
                for si in range(2):
                    skc0, skc1 = j + 1 + 2 * si, j + 2 + 2 * si
                    sc2 = psump.tile([P, 1024], F32, tag="big", bufs=2)
                    nc.tensor.matmul(
                        sc2[:, 0:512],
                        kT[:, skc0 * P : (skc0 + 1) * P],
                        quad_b,
                        start=True,
                        stop=True,
                    )
                    nc.tensor.matmul(
                        sc2[:, 512:1024],
                        kT[:, skc1 * P : (skc1 + 1) * P],
                        quad_b,
                        start=True,
                        stop=True,
                    )
                    at2 = attnp.tile([P, 1024], BF16, tag="at", bufs=12)
                    nc.scalar.activation(
                        at2[:], sc2[:], mybir.ActivationFunctionType.Exp, scale=0.125
                    )
                    if si == 1:
                        nc.vector.tensor_mul(
                            at2[:, 512:1024],
                            at2[:, 512:1024],
                            mkt[:, (j + 4) * 512 : (j + 5) * 512],
                        )
                    ats.append(at2)
                return ats

            def emit_oproj(j):
                # o_proj for pair j from the AllGathered full-channel oT.
                # Readback split in 4 chunk-DMAs on both rings; the two srow
                # PSUM accumulators advance chunk-by-chunk as data lands.
                ag_sb = opartp.tile([P, DC * 256], BF16, tag="agsb", bufs=2)
                for g in range(4):
                    eng = nc.sync if g % 2 == 0 else nc.scalar
                    eng.dma_start(
                        out=ag_sb[:, g * 1024 : (g + 1) * 1024].rearrange(
                            "p (c n) -> p c n", c=4
                        ),
                        in_=agout[j][g * 512 : (g + 1) * 512, :].rearrange(
                            "(c p) n -> p c n", p=P
                        ),
                    )
                po = psump.tile([P, 1024], F32, tag="big", bufs=2, name=f"po{j}")
                for c16 in range(DC):
                    for srow in range(2):
                        nc.tensor.matmul(
                            po[:, srow * 512 : (srow + 1) * 512],
                            ag_sb[:, c16 * 256 + srow * P : c16 * 256 + (srow + 1) * P],
                            wo_all[:, c16 * 512 : (c16 + 1) * 512],
                            start=(c16 == 0),
                            stop=(c16 == DC - 1),
                        )
                for srow in range(2):
                    osb = opartp.tile([P, 512], F32, tag="osb", bufs=3)
                    nc.vector.tensor_copy(osb[:], po[:, srow * 512 : (srow + 1) * 512])
                    nc.sync.dma_start(
                        out=out_p[256 * j + srow * P : 256 * j + (srow + 1) * P, :],
                        in_=osb[:],
                    )

            # ---- software-pipelined attention: scores/exp run one quad
            # ahead of attn@v+normalization so ACT (the pacing engine) never
            # waits on PE and PE stays dense. Pairs run light-first so the
            # serialized AllGather chain starts as early as possible; after
            # each AG trigger the PREVIOUS pair's o_proj is emitted (the
            # gpsimd collective chain guarantees its gather has completed).
            pair_order = (0, 1, 2, 3)
            steps = [(j, qd) for j in pair_order for qd in range(2)]
            prev = None
            for idx, (j, qd) in enumerate(steps):
                if prev is not None:
                    emit_av_norm(*prev)
                ats = emit_scores(j, qd)
                prev = (j, qd, ats)
            emit_av_norm(*prev)
            for j in pair_order:
                emit_oproj(j)

    nc.compile()
    return nc


def _get_nc():
    if "nc" not in _NC_CACHE:
        _NC_CACHE["nc"] = _build_graph()
    return _NC_CACHE["nc"]


def _shard_inputs(x, wq, wk, wv, wo, cos, sin, mask, pos):
    import ml_dtypes

    bf16 = ml_dtypes.bfloat16
    x = np.asarray(x, dtype=np.float32).astype(bf16)
    wq = np.asarray(wq, dtype=np.float32).astype(bf16)
    wk = np.asarray(wk, dtype=np.float32).astype(bf16)
    wv = np.asarray(wv, dtype=np.float32).astype(bf16)
    wo = np.asarray(wo, dtype=np.float32).astype(bf16)
    cos = np.asarray(cos, dtype=np.float32)
    sin = np.asarray(sin, dtype=np.float32)
    mask = np.asarray(mask, dtype=np.float32)
    p = int(pos)

    def pblock(a, nchunks):
        # [(chunks*128), n] -> [128, chunks, n] -> [128, chunks*n]
        n = a.shape[1]
        return np.ascontiguousarray(
            a.reshape(nchunks, P, n).transpose(1, 0, 2).reshape(P, nchunks * n)
        )

    cs = cos[p : p + S]  # [S, 32]
    sn = sin[p : p + S]
    cs8 = pblock(np.tile(cs, (1, NH)), SC).astype(bf16)  # [128, 8*256]
    sn8 = pblock(np.tile(sn, (1, NH)), SC).astype(bf16)
    # transposed diagonal 128x128 blocks of the additive mask, pre-scaled by
    # sqrt(HD) so exp(scale*(scores + 8*mask)) == exp(scores/8 + mask)
    mkb = np.concatenate(
        [
            np.tile(
                (mask[j * P : (j + 1) * P, j * P : (j + 1) * P].T >= -0.5).astype(
                    bf16
                ),
                (1, 4),
            )
            for j in range(SC)
        ],
        axis=1,
    )
    mkb = np.ascontiguousarray(mkb)  # [128, 8*512], diag blocks tiled x4 heads

    in_maps = []
    for d in range(N_CORES):
        g, kv = d // 4, d % 4
        in_maps.append(
            {
                "xt": np.ascontiguousarray(
                    x[g].T.reshape(DC, P, SC, P).transpose(2, 1, 0, 3).reshape(SC, P, D)
                ),
                "wq": pblock(wq[:, kv * 512 : (kv + 1) * 512], DC),
                "wkv": pblock(
                    np.concatenate(
                        [
                            wk[:, kv * HD : (kv + 1) * HD],
                            wv[:, kv * HD : (kv + 1) * HD],
                        ],
                        axis=1,
                    ),
                    DC,
                ),
                "wo": pblock(wo[:, kv * 512 : (kv + 1) * 512], DC),
                "cs8": cs8,
                "sn8": sn8,
                "mkb": mkb,
            }
        )
    return in_maps


def _run(inputs, trace=False, trace_kwargs=None):
    nc = _get_nc()
    in_maps = _shard_inputs(**inputs)
    res = run_bass_kernel_spmd(
        nc,
        in_maps,
        core_ids=list(range(N_CORES)),
        trace=trace,
        **(trace_kwargs or {}),
    )
    B = 2
    out = np.empty((B, S, D), dtype=np.float32)
    for d in range(N_CORES):
        g, kv = d // 4, d % 4
        core_out = res.results[d]["out"]  # [1024, 512]; rows 256j.. = pair j
        cols = slice(kv * 512, (kv + 1) * 512)
        for j in range(4):
            out[g, j * P : (j + 1) * P, cols] = core_out[256 * j : 256 * j + P]
            out[g, (j + 4) * P : (j + 5) * P, cols] = core_out[
                256 * j + P : 256 * j + 256
            ]
    return out, res


def kernel(**inputs) -> np.ndarray:
    out, _ = _run(inputs, trace=False)
    return out



# revision 31
# speedup vs baseline: 1.0801x; 1.0446x over previous
# Distributed GQA attention prefill kernel for one TRN2 chip (8 NeuronCores).
#
# Problem: B=2, S=1024, D=2048, H=32 q-heads, KV=4 kv-heads, HD=64, causal,
# RoPE, f32 I/O. Sharding: core d = (batch g=d//4, kv-head kv=d%4). Each core
# computes q-proj for its 8 q heads, k/v-proj for its kv head, attention for
# 8 heads, and the full-channel o_proj for its 512 output columns over 256-row
# blocks, fed by one bf16 AllGather of oT channels per q-block pair (j, j+4)
# across its group of 4 cores.
#
# Key structure (from perfetto-driven tuning):
# - All inputs pre-cast to bf16 on the host; plain HWDGE DMAs, xt first.
# - Attention is software-pipelined per head-quad step: score matmuls + one
#   fused [128,1024] exp (2 PSUM banks) run one step ahead of attn@v, so ACT
#   (the pacing engine at ~1.15us/exp) never waits and PE stays dense.
# - vaug carries 64 ones-rows, so attn@v emits softmax denominators
#   pre-broadcast 64-ways: normalization is one [64,512] copy + one
#   reciprocal_approx_fast + 4 muls per block, no [1,N] single-lane ops.
# - Pairs run light-first (0,1,2,3); AllGathers serialize on the gpsimd
#   collective chain, so o_proj(pair k-2) is emitted after AG(k) triggers
#   (its gather is then provably complete) and the last AG overlaps the two
#   remaining o_projs.
import sys

import numpy as np

try:
    import concourse.bass as bass  # noqa: F401
except ImportError:
    for p in ("/opt/trn_rl_repo", "/root/.axon_site/_ro/trn_rl_repo"):
        if p not in sys.path:
            sys.path.append(p)
    import concourse.bass as bass  # noqa: F401

import concourse.bacc as bacc
import concourse.mybir as mybir
import concourse.tile as tile
from concourse import masks
from concourse.bass_utils import run_bass_kernel_spmd

S = 1024
D = 2048
H = 32
KV = 4
HD = 64
NH = 8  # q heads per core
P = 128
SC = S // P  # 8 seq chunks
DC = D // P  # 16 D chunks
N_CORES = 8
GROUPS = [[0, 1, 2, 3], [4, 5, 6, 7]]

F32 = mybir.dt.float32
BF16 = mybir.dt.bfloat16

_NC_CACHE = {}


def _build_graph():
    nc = bacc.Bacc("TRN2", target_bir_lowering=False, debug=False, num_devices=N_CORES)

    xt_p = nc.dram_tensor("xt", [SC, P, DC * P], BF16, kind="ExternalInput")
    wq_p = nc.dram_tensor("wq", [P, DC * 512], BF16, kind="ExternalInput")
    wkv_p = nc.dram_tensor("wkv", [P, DC * 2 * HD], BF16, kind="ExternalInput")
    wo_p = nc.dram_tensor("wo", [P, DC * 512], BF16, kind="ExternalInput")
    cs_p = nc.dram_tensor("cs8", [P, SC * 256], BF16, kind="ExternalInput")
    sn_p = nc.dram_tensor("sn8", [P, SC * 256], BF16, kind="ExternalInput")
    mk_p = nc.dram_tensor("mkb", [P, SC * 512], BF16, kind="ExternalInput")
    out_p = nc.dram_tensor("out", [S, 512], F32, kind="ExternalOutput")

    with tile.TileContext(nc) as tc:
        with (
            tc.tile_pool(name="const", bufs=1) as constp,
            tc.tile_pool(name="big", bufs=1) as bigp,
            tc.tile_pool(name="work", bufs=1) as workp,
            tc.tile_pool(name="rt", bufs=4) as rtp,
            tc.tile_pool(name="attn", bufs=3) as attnp,
            tc.tile_pool(name="opart", bufs=2) as opartp,
            tc.tile_pool(name="tiny", bufs=1) as tinyp,
            tc.tile_pool(name="psum", bufs=1, space="PSUM") as psump,
            tc.tile_pool(name="dram", bufs=1, space="DRAM") as dramp,
        ):
            # ---- constants (cheap engine work first; DMAs ordered by need) ----
            ident = constp.tile([P, P], BF16, tag="ident")
            masks.make_identity(nc, ident[:])

            # Bulk loads: inputs are pre-cast to bf16 on the host, so these
            # are plain direct DMAs on the two HWDGE rings — no staging.
            xT_all = bigp.tile([P, SC * DC * P], BF16, tag="xT_all")
            wkv_all = bigp.tile([P, DC * 2 * HD], BF16, tag="wkv_all")
            wq_all = bigp.tile([P, DC * 512], BF16, tag="wq_all")
            wo_all = bigp.tile([P, DC * 512], BF16, tag="wo_all")

            def load_xt(s, eng):
                eng.dma_start(
                    out=xT_all[:, s * D : (s + 1) * D], in_=xt_p[s, :, :]
                )

            def load_wq(g, eng):
                eng.dma_start(
                    out=wq_all[:, g * 2048 : (g + 1) * 2048],
                    in_=wq_p[:, g * 2048 : (g + 1) * 2048],
                )

            def load_wo(g, eng):
                eng.dma_start(
                    out=wo_all[:, g * 2048 : (g + 1) * 2048],
                    in_=wo_p[:, g * 2048 : (g + 1) * 2048],
                )

            cst = constp.tile([P, SC * 256], BF16, tag="cst")  # cos, tiled x8 heads
            snt = constp.tile([P, SC * 256], BF16, tag="snt")
            mkt = constp.tile([P, SC * 512], BF16, tag="mkt")  # binary diag masks^T
            nc.scalar.dma_start(out=wkv_all[:], in_=wkv_p[:, :])
            for s in range(SC):
                load_xt(s, nc.sync if s % 2 == 0 else nc.scalar)
            nc.sync.dma_start(out=cst[:], in_=cs_p[:, :])
            nc.scalar.dma_start(out=snt[:], in_=sn_p[:, :])
            for g in range(4):
                load_wq(g, nc.sync if g % 2 == 0 else nc.scalar)
            nc.sync.dma_start(out=mkt[:], in_=mk_p[:, :])
            for g in range(4):
                load_wo(g, nc.scalar if g % 2 == 0 else nc.sync)

            # ---- projections + RoPE (natural [s, ch] layout) ----
            qrot = [workp.tile([P, NH * HD], BF16, tag=f"qr{s}", name=f"qr{s}") for s in range(SC)]
            krot = [workp.tile([P, HD], BF16, tag=f"kr{s}", name=f"kr{s}") for s in range(SC)]
            vaug = [workp.tile([P, 2 * HD], BF16, tag=f"va{s}", name=f"va{s}") for s in range(SC)]

            def rope(ps_ap, dst, s, nh):
                # ps_ap: PSUM AP [128, nh*64] f32; dst: SBUF bf16 same shape
                pv = ps_ap.rearrange("p (h t c) -> p h t c", h=nh, t=2)
                dv = dst[:].rearrange("p (h t c) -> p h t c", h=nh, t=2)
                cs = cst[:, s * 256 : s * 256 + nh * 32].rearrange(
                    "p (h c) -> p h c", h=nh
                )
                sn = snt[:, s * 256 : s * 256 + nh * 32].rearrange(
                    "p (h c) -> p h c", h=nh
                )
                lo, hi = pv[:, :, 0, :], pv[:, :, 1, :]
                t1 = rtp.tile([P, NH * 32], F32, tag="rt1")
                t2 = rtp.tile([P, NH * 32], F32, tag="rt2")
                t1v = t1[:, : nh * 32].rearrange("p (h c) -> p h c", h=nh)
                t2v = t2[:, : nh * 32].rearrange("p (h c) -> p h c", h=nh)
                nc.any.tensor_mul(t1v, lo, cs)
                nc.any.tensor_mul(t2v, hi, sn)
                nc.any.tensor_sub(dv[:, :, 0, :], t1v, t2v)
                nc.any.tensor_mul(t1v, hi, cs)
                nc.any.tensor_mul(t2v, lo, sn)
                nc.any.tensor_add(dv[:, :, 1, :], t1v, t2v)

            qTall = workp.tile([64, NH * S], BF16, tag="qTall")
            kT = workp.tile([64, S], BF16, tag="kT")

            def emit_kv(s):
                pkv_t = psump.tile([P, 1024], F32, tag="big", bufs=2, name=f"pkv{s}")
                pkv = pkv_t[:, 0 : 2 * HD]
                for d in range(DC):
                    nc.tensor.matmul(
                        pkv,
                        xT_all[:, s * D + d * P : s * D + (d + 1) * P],
                        wkv_all[:, d * 2 * HD : (d + 1) * 2 * HD],
                        start=(d == 0),
                        stop=(d == DC - 1),
                    )
                rope(pkv[:, 0:HD], krot[s], s, 1)
                nc.scalar.copy(vaug[s][:, 0:HD], pkv[:, HD : 2 * HD])
                nc.any.memset(vaug[s][:, HD : 2 * HD], 1.0)
                tpk = psump.tile([64, P], BF16, tag="oa", bufs=2, name=f"tpk{s}")
                nc.tensor.transpose(tpk[:], krot[s][:], ident[:])
                nc.scalar.copy(kT[:, s * P : (s + 1) * P], tpk[:])

            def emit_q(s):
                pq_t = psump.tile([P, 1024], F32, tag="big", bufs=2, name=f"pq{s}")
                pq = pq_t[:, 0 : NH * HD]
                for d in range(DC):
                    nc.tensor.matmul(
                        pq,
                        xT_all[:, s * D + d * P : s * D + (d + 1) * P],
                        wq_all[:, d * 512 : (d + 1) * 512],
                        start=(d == 0),
                        stop=(d == DC - 1),
                    )
                rope(pq[:, :], qrot[s], s, NH)
                for h in range(NH):
                    tpq = psump.tile([64, P], BF16, tag="oa", bufs=2, name=f"tpq{s}_{h}")
                    nc.tensor.transpose(
                        tpq[:], qrot[s][:, h * HD : (h + 1) * HD], ident[:]
                    )
                    nc.scalar.copy(
                        qTall[:, h * S + s * P : h * S + (s + 1) * P], tpq[:]
                    )

            for s in range(SC):
                emit_kv(s)
            for s in range(SC):
                emit_q(s)

            # view: [64, a(4), c(2), h(8), i(128)]; a scores matmul's rhs takes
            # (c, h, i)-ordered columns: [j:h | j:h' | j+4:h | j+4:h']
            qview = qTall[:].rearrange("p (h c a b) -> p a c h b", h=NH, c=2, a=4, b=P)

            # oTp[j][c]: pair-j output tile for channel group c (2 heads),
            # cols = [block j (128) | block j+4 (128)] -> contiguous AG ship.
            oTp = [
                [
                    workp.tile([P, 256], BF16, tag=f"oTp{j}c{c}", name=f"oTp{j}c{c}")
                    for c in range(4)
                ]
                for j in range(4)
            ]
            agin = [dramp.tile([512, 256], BF16, name=f"agin{j}") for j in range(4)]
            agout = [dramp.tile([D, 256], BF16, name=f"agout{j}") for j in range(4)]

            def emit_av_norm(j, qd, ats):
                """attn@v for quad (j, qd) into one fused [128, 1024] PSUM
                (block j in cols 0:512, block j+4 in 512:1024). Rows 0:64 are
                o, rows 64:128 are the softmax denominators pre-broadcast 64
                ways (vaug cols 64:128 are ones), so normalization needs no
                [1,N] single-lane ops and no broadcast matmul."""
                oa = psump.tile([P, 1024], F32, tag="oa", bufs=2)
                for skc in range(j + 1):  # block j accumulation
                    nc.tensor.matmul(
                        oa[:, 0:512],
                        vaug[skc][:],
                        ats[skc][:, 0:512],
                        start=(skc == 0),
                        stop=(skc == j),
                        skip_group_check=True,
                    )
                for skc in range(j + 5):  # block j+4 accumulation
                    if skc <= j:
                        rhs = ats[skc][:, 512:1024]
                    else:
                        si, half = divmod(skc - j - 1, 2)
                        rhs = ats[j + 1 + si][:, half * 512 : half * 512 + 512]
                    nc.tensor.matmul(
                        oa[:, 512:1024],
                        vaug[skc][:],
                        rhs,
                        start=(skc == 0),
                        stop=(skc == j + 4),
                        skip_group_check=True,
                    )
                # per-half normalization chains (block a overlaps block b's AV)
                for bi, qb in enumerate((j, j + 4)):
                    cols = slice(bi * 512, bi * 512 + 512)
                    rec = tinyp.tile([64, 512], F32, tag=f"rec{bi}")
                    nc.vector.tensor_copy(rec[:], oa[HD : 2 * HD, cols])
                    rc2 = tinyp.tile([64, 512], F32, tag=f"rc2{bi}")
                    nc.vector.reciprocal_approx_fast(rc2[:], rec[:])
                    for h4 in range(4):
                        h = 4 * qd + h4
                        nc.vector.tensor_mul(
                            oTp[j][h // 2][
                                64 * (h % 2) : 64 * (h % 2) + 64,
                                bi * P : (bi + 1) * P,
                            ],
                            oa[0:HD, bi * 512 + h4 * P : bi * 512 + (h4 + 1) * P],
                            rc2[:, h4 * P : (h4 + 1) * P],
                        )
                # ship the two channel groups this quad completed
                for c in (2 * qd, 2 * qd + 1):
                    nc.scalar.dma_start(
                        out=agin[j][c * P : (c + 1) * P, :], in_=oTp[j][c][:]
                    )
                if qd == 1:
                    nc.gpsimd.collective_compute(
                        "AllGather",
                        mybir.AluOpType.bypass,
                        replica_groups=GROUPS,
                        ins=[agin[j].opt()],
                        outs=[agout[j].opt()],
                    )

            def emit_scores(j, qd):
                """score matmuls + fused exps for quad (j, qd); returns the
                list of at tiles: ats[0..j] are both-tiles, ats[j+1], ats[j+2]
                are the fused single pairs (skc j+1/j+2 and j+3/j+4)."""
                quad = qview[:, j, 0, 4 * qd : 4 * qd + 4, :]
                quad_b = qview[:, j, 1, 4 * qd : 4 * qd + 4, :]
                ats = []
                for skc in range(j + 1):
                    sc2 = psump.tile([P, 1024], F32, tag="big", bufs=2)
                    nc.tensor.matmul(
                        sc2[:, 0:512],
                        kT[:, skc * P : (skc + 1) * P],
                        quad,
                        start=True,
                        stop=True,
                    )
                    nc.tensor.matmul(
                        sc2[:, 512:1024],
                        kT[:, skc * P : (skc + 1) * P],
                        quad_b,
                        start=True,
                        stop=True,
                    )
                    at2 = attnp.tile([P, 1024], BF16, tag="at", bufs=12)
                    nc.scalar.activation(
                        at2[:], sc2[:], mybir.ActivationFunctionType.Exp, scale=0.125
                    )
                    if skc == j:
                        nc.vector.tensor_mul(
                            at2[:, 0:512],
                            at2[:, 0:512],
                            mkt[:, j * 512 : (j + 1) * 512],
                        )
                    ats.append(at2)
                for si in range(2):
                    skc0, skc1 = j + 1 + 2 * si, j + 2 + 2 * si
                    sc2 = psump.tile([P, 1024], F32, tag="big", bufs=2)
                    nc.tensor.matmul(
                        sc2[:, 0:512],
                        kT[:, skc0 * P : (skc0 + 1) * P],
                        quad_b,
                        start=True,
                        stop=True,
                    )
                    nc.tensor.matmul(
                        sc2[:, 512:1024],
                        kT[:, skc1 * P : (skc1 + 1) * P],
                        quad_b,
                        start=True,
                        stop=True,
                    )
                    at2 = attnp.tile([P, 1024], BF16, tag="at", bufs=12)
                    nc.scalar.activation(
                        at2[:], sc2[:], mybir.ActivationFunctionType.Exp, scale=0.125
                    )
                    if si == 1:
                        nc.vector.tensor_mul(
                            at2[:, 512:1024],
                            at2[:, 512:1024],
                            mkt[:, (j + 4) * 512 : (j + 5) * 512],
                        )
                    ats.append(at2)
                return ats

            def emit_oproj(j):
                # o_proj for pair j from the AllGathered full-channel oT.
                # Readback split in 4 chunk-DMAs on both rings; the two srow
                # PSUM accumulators advance chunk-by-chunk as data lands.
                ag_sb = opartp.tile([P, DC * 256], BF16, tag="agsb", bufs=2)
                for g in range(4):
                    eng = nc.sync if g % 2 == 0 else nc.scalar
                    eng.dma_start(
                        out=ag_sb[:, g * 1024 : (g + 1) * 1024].rearrange(
                            "p (c n) -> p c n", c=4
                        ),
                        in_=agout[j][g * 512 : (g + 1) * 512, :].rearrange(
                            "(c p) n -> p c n", p=P
                        ),
                    )
                po = psump.tile([P, 1024], F32, tag="big", bufs=2, name=f"po{j}")
                for c16 in range(DC):
                    for srow in range(2):
                        nc.tensor.matmul(
                            po[:, srow * 512 : (srow + 1) * 512],
                            ag_sb[:, c16 * 256 + srow * P : c16 * 256 + (srow + 1) * P],
                            wo_all[:, c16 * 512 : (c16 + 1) * 512],
                            start=(c16 == 0),
                            stop=(c16 == DC - 1),
                        )
                for srow in range(2):
                    osb = opartp.tile([P, 512], F32, tag="osb", bufs=3)
                    nc.vector.tensor_copy(osb[:], po[:, srow * 512 : (srow + 1) * 512])
                    nc.sync.dma_start(
                        out=out_p[256 * j + srow * P : 256 * j + (srow + 1) * P, :],
                        in_=osb[:],
                    )

            # ---- software-pipelined attention: scores/exp run one quad
            # ahead of attn@v+normalization so ACT (the pacing engine) never
            # waits on PE and PE stays dense. Pairs run light-first so the
            # serialized AllGather chain starts as early as possible; after
            # each AG trigger the PREVIOUS pair's o_proj is emitted (the
            # gpsimd collective chain guarantees its gather has completed).
            pair_order = (0, 1, 2, 3)
            steps = [(j, qd) for j in pair_order for qd in range(2)]
            prev = None
            for idx, (j, qd) in enumerate(steps):
                if prev is not None:
                    emit_av_norm(*prev)
                ats = emit_scores(j, qd)
                prev = (j, qd, ats)
            emit_av_norm(*prev)
            for j in pair_order:
                emit_oproj(j)

    nc.compile()
    return nc


def _get_nc():
    if "nc" not in _NC_CACHE:
        _NC_CACHE["nc"] = _build_graph()
    return _NC_CACHE["nc"]


def _shard_inputs(x, wq, wk, wv, wo, cos, sin, mask, pos):
    import ml_dtypes

    bf16 = ml_dtypes.bfloat16
    x = np.asarray(x, dtype=np.float32).astype(bf16)
    wq = np.asarray(wq, dtype=np.float32).astype(bf16)
    wk = np.asarray(wk, dtype=np.float32).astype(bf16)
    wv = np.asarray(wv, dtype=np.float32).astype(bf16)
    wo = np.asarray(wo, dtype=np.float32).astype(bf16)
    cos = np.asarray(cos, dtype=np.float32)
    sin = np.asarray(sin, dtype=np.float32)
    mask = np.asarray(mask, dtype=np.float32)
    p = int(pos)

    def pblock(a, nchunks):
        # [(chunks*128), n] -> [128, chunks, n] -> [128, chunks*n]
        n = a.shape[1]
        return np.ascontiguousarray(
            a.reshape(nchunks, P, n).transpose(1, 0, 2).reshape(P, nchunks * n)
        )

    cs = cos[p : p + S]  # [S, 32]
    sn = sin[p : p + S]
    cs8 = pblock(np.tile(cs, (1, NH)), SC).astype(bf16)  # [128, 8*256]
    sn8 = pblock(np.tile(sn, (1, NH)), SC).astype(bf16)
    # transposed diagonal 128x128 blocks of the additive mask, pre-scaled by
    # sqrt(HD) so exp(scale*(scores + 8*mask)) == exp(scores/8 + mask)
    mkb = np.concatenate(
        [
            np.tile(
                (mask[j * P : (j + 1) * P, j * P : (j + 1) * P].T >= -0.5).astype(
                    bf16
                ),
                (1, 4),
            )
            for j in range(SC)
        ],
        axis=1,
    )
    mkb = np.ascontiguousarray(mkb)  # [128, 8*512], diag blocks tiled x4 heads

    in_maps = []
    for d in range(N_CORES):
        g, kv = d // 4, d % 4
        in_maps.append(
            {
                "xt": np.ascontiguousarray(
                    x[g].T.reshape(DC, P, SC, P).transpose(2, 1, 0, 3).reshape(SC, P, D)
                ),
                "wq": pblock(wq[:, kv * 512 : (kv + 1) * 512], DC),
                "wkv": pblock(
                    np.concatenate(
                        [
                            wk[:, kv * HD : (kv + 1) * HD],
                            wv[:, kv * HD : (kv + 1) * HD],
                        ],
                        axis=1,
                    ),
                    DC,
                ),
                "wo": pblock(wo[:, kv * 512 : (kv + 1) * 512], DC),
                "cs8": cs8,
                "sn8": sn8,
                "mkb": mkb,
            }
        )
    return in_maps


def _run(inputs, trace=False, trace_kwargs=None):
    nc = _get_nc()
    in_maps = _shard_inputs(**inputs)
    res = run_bass_kernel_spmd(
        nc,
        in_maps,
        core_ids=list(range(N_CORES)),
        trace=trace,
        **(trace_kwargs or {}),
    )
    B = 2
    out = np.empty((B, S, D), dtype=np.float32)
    for d in range(N_CORES):
        g, kv = d // 4, d % 4
        core_out = res.results[d]["out"]  # [1024, 512]; rows 256j.. = pair j
        cols = slice(kv * 512, (kv + 1) * 512)
        for j in range(4):
            out[g, j * P : (j + 1) * P, cols] = core_out[256 * j : 256 * j + P]
            out[g, (j + 4) * P : (j + 5) * P, cols] = core_out[
                256 * j + P : 256 * j + 256
            ]
    return out, res


def kernel(**inputs) -> np.ndarray:
    out, _ = _run(inputs, trace=False)
    return out

